# revision 28
# baseline (speedup 1.0000x reference)
"""AnomalyDAE 4-layer GCN on 8 TRN2 NeuronCores.

Strategy (node sharding):
  - Nodes partitioned contiguously across 8 cores (6250/core).
  - Per layer: local matmul h = A @ W (PE, bf16 in / f32 acc), pre-scaled by
    dinv so symmetric normalization becomes out = dinv * segsum(hs[src]),
    hs = dinv * h.  AllGather the bf16 hs table -> [N, dout] per core.
  - Message passing: edges (incl. self-loops) sorted by (dst tile, table
    half, dst 64-half) on host.  Per 128-token block: dma_gather the src
    rows (bf16), build a 64-column one-hot Seg[tok, row64] =
    (dst_rel[tok] == iota) on DVE, and matmul Seg^T @ msg accumulates the
    tile's [64, dout] half-result in PSUM (rows 0:64 / 64:128 are two
    independent accumulation groups).  64-wide one-hot halves both DVE
    generation work and PE LDWEIGHTS columns vs. a 128-wide one.
  - Gather indices (int16, 16-partition wrap) and dst_rel are identical
    across layers: preloaded into SBUF once, no per-call index DMAs.
  - The next layer's local matmul is fused into the aggregation epilogue
    (relu -> PE-transpose -> matmul -> dinv scale -> hs dma).  AllGather
    half-A fires after tile 24's hs write (mid-layer), half-B after the
    last tile, so the collectives overlap the previous layer's tail and
    the gather queues never starve at layer boundaries.
  - int16 gather indices => the feature table is split in two halves (per
    core: rows [0,3200) -> table A, rest -> table B), each AllGathered
    separately.  The d=64 layer is zero-padded to 128 features (gather
    element size must be a multiple of 256 bytes).
"""

import math
import os

import numpy as np

N_CORES = 8
P = 128
GROUP = 1  # dst tiles per gather call pair

_CACHE = {}
LAST_EXEC_NS = None


def _chunk_splits(ntiles):
    """Tile counts per table chunk: two big chunks + a small tail chunk so
    the layer-boundary collective only covers the tail's tokens."""
    if ntiles < 3:
        return [ntiles]
    a = max(1, (2 * ntiles) // 5)
    return [a, a, ntiles - 2 * a]


def _build_gcn(n_nodes, npc, npad, dims, layout, flags01, total_tok, n_cores):
    import concourse.bacc as bacc
    import concourse.tile as tile
    from concourse import mybir
    from concourse.library_config import mlp
    from contextlib import ExitStack

    f32 = mybir.dt.float32
    bf16 = mybir.dt.bfloat16
    i16 = mybir.dt.int16
    ntiles = npad // P
    single_packet = os.environ.get("GCN_SP", "1") == "1"
    # NOTE: Tile locks each DMASW semaphore to the SWDGE queue that first
    # uses it (sems assigned round-robin in emission order), so the queue
    # cycle must be periodic with period 4.
    QCYC = [int(c) for c in os.environ.get("GCN_QCYC", "1230")]

    nc = bacc.Bacc(
        "TRN2", debug=False, num_devices=n_cores, num_swdge_queues=4
    )

    din0 = dims[0][0]
    kc0 = math.ceil(din0 / P)
    xT = nc.declare_dram_parameter("xT", [din0, npad], bf16, isOutput=False)
    Ws = [
        nc.declare_dram_parameter(f"w{i}", [din, dout], bf16, isOutput=False)
        for i, (din, dout) in enumerate(dims)
    ]
    Bs = [
        nc.declare_dram_parameter(f"b{i}", [P, dout], f32, isOutput=False)
        for i, (din, dout) in enumerate(dims)
    ]
    dinv_in = nc.declare_dram_parameter("dinv", [P, ntiles], f32, isOutput=False)
    ident_in = nc.declare_dram_parameter("ident", [P, P], bf16, isOutput=False)
    nbmax = max(
        (j1 - j0)
        for grp in layout
        for _, branges in grp["tiles"]
        for _, j0, j1 in branges
    )
    riota_in = [
        nc.declare_dram_parameter(f"riota{d}", [P, nbmax, 64], bf16, isOutput=False)
        for d in range(2)
    ]
    gi_in = nc.declare_dram_parameter(
        "gidx", [P, total_tok // 16], i16, isOutput=False
    )
    dmax = max(d for _, d in dims)
    drel_in = nc.declare_dram_parameter(
        "drel", [P, total_tok // P], bf16, isOutput=False
    )
    dout_last = dims[-1][1]
    out_ext = nc.declare_dram_parameter("out", [npc, dout_last], f32, isOutput=True)

    # Table split into chunks by tile ranges; each chunk AllGathered as soon
    # as its tiles' local matmuls are done, so only the (small) last chunk's
    # collective sits on the layer boundary.
    csplit = _chunk_splits(ntiles)
    cstart = [sum(csplit[:c]) for c in range(len(csplit))]
    crows = [
        min((cstart[c] + csplit[c]) * P, npc) - cstart[c] * P
        for c in range(len(csplit))
    ]
    nch = len(csplit)
    hs_loc = [
        tuple(
            nc.dram_tensor(f"hs_loc{c}_{i}", [crows[c], d], bf16)
            for c in range(nch)
        )
        for i, (_, d) in enumerate(dims)
    ]
    hs_full = [
        tuple(
            nc.dram_tensor(
                f"hs_full{c}_{i}",
                [n_cores * crows[c], d],
                bf16,
                addr_space="Shared",
            )
            for c in range(nch)
        )
        for i, (_, d) in enumerate(dims)
    ]

    relu = mybir.ActivationFunctionType.Relu
    copyf = mybir.ActivationFunctionType.Copy
    mult = mybir.AluOpType.mult
    add = mybir.AluOpType.add
    iseq = mybir.AluOpType.is_equal

    n_layers = len(dims)

    with tile.TileContext(nc) as tc, ExitStack() as ctx:
        const = ctx.enter_context(tc.tile_pool(name="const", bufs=1))
        at_pool = ctx.enter_context(tc.tile_pool(name="acts", bufs=2))
        work = ctx.enter_context(tc.tile_pool(name="work", bufs=4))
        msgp = ctx.enter_context(tc.tile_pool(name="msg", bufs=6))
        segp = ctx.enter_context(tc.tile_pool(name="seg", bufs=10))
        idxp = ctx.enter_context(tc.tile_pool(name="idx", bufs=6))
        psum = ctx.enter_context(tc.tile_pool(name="psum", bufs=2, space="PSUM"))
        psacc = ctx.enter_context(tc.tile_pool(name="psacc", bufs=4, space="PSUM"))

        nc.gpsimd.load_library(mlp)

        ident = const.tile([P, P], bf16)
        nc.sync.dma_start(out=ident[:], in_=ident_in[:, :])
        riota = []
        for d in range(2):
            rt = const.tile([P, nbmax, 64], bf16, tag=f"riota{d}")
            nc.sync.dma_start(out=rt[:], in_=riota_in[d][:, :, :])
            riota.append(rt)
        dinv_sb = const.tile([P, ntiles], f32)
        nc.sync.dma_start(out=dinv_sb[:], in_=dinv_in[:])
        drel_sb = const.tile([P, total_tok // P], bf16)
        nc.sync.dma_start(out=drel_sb[:], in_=drel_in[:, :])
        # persistent per-tile local hs (self-loop term, added in the epilogue)
        hskeep = const.tile([P, ntiles, dmax], bf16)

        w_sb, b_sb = [], []
        for i, (din, dout) in enumerate(dims):
            kcs = math.ceil(din / P)
            wi = []
            for kc in range(kcs):
                rows = min(P, din - kc * P)
                wt = const.tile([rows, dout], bf16, tag=f"w{i}_{kc}")
                nc.sync.dma_start(out=wt[:], in_=Ws[i][kc * P : kc * P + rows, :])
                wi.append(wt)
            w_sb.append(wi)
            bt = const.tile([P, dout], f32, tag=f"b{i}")
            nc.sync.dma_start(out=bt[:], in_=Bs[i][:, :])
            b_sb.append(bt)

        def local_mm(li, t, aT):
            """h = a @ W[li] for shard tile t, dinv-prescale, dma to hs_loc."""
            dout = dims[li][1]
            ps = psum.tile([P, dout], f32, tag="mm", name="mm")
            kcs = len(aT)
            for kc in range(kcs):
                nc.tensor.matmul(
                    ps[:],
                    aT[kc][:, t * P : (t + 1) * P],
                    w_sb[li][kc][:],
                    start=(kc == 0),
                    stop=(kc == kcs - 1),
                )
            hs_t = hskeep[:, t, 0:dout]
            nc.scalar.activation(
                hs_t, ps[:], copyf, bias=0.0, scale=dinv_sb[:, t : t + 1]
            )
            ch = next(c for c in range(nch) if t < cstart[c] + csplit[c])
            r0 = (t - cstart[ch]) * P
            r1 = min(r0 + P, crows[ch])
            nc.sync.dma_start(
                out=hs_loc[li][ch][r0:r1, :], in_=hskeep[0 : r1 - r0, t, 0:dout]
            )

        def fire_collective(li, ch):
            nc.gpsimd.collective_compute(
                "AllGather",
                mybir.AluOpType.bypass,
                replica_groups=[list(range(n_cores))],
                ins=[hs_loc[li][ch][:, :]],
                outs=[hs_full[li][ch][:, :]],
            )

        fire_at = {cstart[c] + csplit[c] - 1: c for c in range(nch)}

        # ---- layer-0 lead-in: load xT, local matmul per tile, AllGather ----
        aT = []
        for kc in range(kc0):
            rows = min(P, din0 - kc * P)
            t_ = at_pool.tile([rows, npad], bf16, tag=f"aT_{kc}", name=f"aT0_{kc}")
            nc.sync.dma_start(out=t_[:], in_=xT[kc * P : kc * P + rows, :])
            aT.append(t_)
        for t in range(ntiles):
            local_mm(0, t, aT)
            if t in fire_at:
                fire_collective(0, fire_at[t])

        qn = 0
        for li, (din, dout) in enumerate(dims):
            last = li == n_layers - 1

            if not last:
                dnext = dims[li + 1][0]
                kcs_next = math.ceil(dnext / P)
                aT_next = []
                for kc in range(kcs_next):
                    rows = min(P, dnext - kc * P)
                    aT_next.append(
                        at_pool.tile(
                            [rows, npad],
                            bf16,
                            tag=f"aT_{kc}",
                            name=f"aT{li + 1}_{kc}",
                        )
                    )

            for grp in layout:
                msgs = {}
                for h, c0, c1 in grp["calls"]:
                    # weighted round-robin: queues 1/2 generate descriptors
                    # ~1.7x faster than 0/3 (HW-measured), so feed them more
                    q = QCYC[qn % len(QCYC)]
                    qn += 1
                    ntok = c1 - c0
                    idxt = idxp.tile([P, ntok // 16], i16, tag="idxt", name="idxt")
                    nc.sync.dma_start(out=idxt[:], in_=gi_in[:, c0 // 16 : c1 // 16])
                    msg = msgp.tile([P, ntok // P, dout], bf16, tag="msg", name="msg")
                    nc.gpsimd.dma_gather(
                        msg[:],
                        hs_full[li][h][:, :],
                        idxt[:],
                        ntok,
                        ntok,
                        dout,
                        single_packet=single_packet,
                        queue_num=q,
                    )
                    msgs[h] = (msg, c0 // P)

                for t, branges in grp["tiles"]:
                    pacc = psacc.tile([P, dout], f32, tag="segacc", name="segacc")
                    # Per (tile, h) brange, half-0 blocks form a prefix and
                    # half-1 blocks a suffix (tokens sorted by dst 64-half);
                    # the straddle block appears in both.  Two independent
                    # PSUM accumulation groups: rows 0:64 (array cols 0:64)
                    # and 64:128 (cols 64:128).  MMs of the two groups are
                    # interleaved so each LDWEIGHTS targets the idle column
                    # strip and overlaps the other strip's MATMUL.
                    rngs = []  # (h, d64, b0, b1)
                    for h, j0, j1 in branges:
                        f0 = [j for j in range(j0, j1) if flags01[j][0]]
                        f1 = [j for j in range(j0, j1) if flags01[j][1]]
                        if f0:
                            rngs.append((h, 0, f0[0], f0[-1] + 1))
                        if f1:
                            rngs.append((h, 1, f1[0], f1[-1] + 1))
                    mmseq = [[], []]  # per d64: (seg_tile, blk_in_seg, h, j)
                    for h, d64, b0, b1 in rngs:
                        nblk = b1 - b0
                        sg = segp.tile([P, nblk, 64], bf16, tag="seg", name="seg")
                        nc.vector.tensor_tensor(
                            out=sg[:],
                            in0=drel_sb[:, b0:b1].to_broadcast([P, nblk, 64]),
                            in1=riota[d64][:, 0:nblk, :],
                            op=iseq,
                        )
                        for j in range(b0, b1):
                            mmseq[d64].append((sg, j - b0, h, j))
                    tot = [len(mmseq[0]), len(mmseq[1])]
                    nmm = [0, 0]
                    order = []
                    for k in range(max(tot)):
                        for d64 in range(2):
                            if k < tot[d64]:
                                order.append((d64, k))
                    for d64, k in order:
                        sg, bi, h, j = mmseq[d64][k]
                        msg, base = msgs[h]
                        nc.tensor.matmul(
                            pacc[d64 * 64 : d64 * 64 + 64, :],
                            sg[:, bi, :],
                            msg[:, j - base, :],
                            start=(nmm[d64] == 0),
                            stop=(nmm[d64] == tot[d64] - 1),
                            skip_group_check=True,
                        )
                        nmm[d64] += 1

                    # ---- epilogue straight from PSUM ----
                    # y = dinv*(pacc) + (dinv*hs_self + b); self-loop term
                    # comes from the persistent local hs tile, not a gather.
                    ys = work.tile([P, dout], f32, tag="ys", name="ys")
                    nc.vector.scalar_tensor_tensor(
                        ys[:],
                        hskeep[:, t, 0:dout],
                        dinv_sb[:, t : t + 1],
                        b_sb[li][:],
                        mult,
                        add,
                    )
                    y = work.tile([P, dout], f32, tag="y", name="y")
                    nc.vector.scalar_tensor_tensor(
                        y[:], pacc[:], dinv_sb[:, t : t + 1], ys[:], mult, add
                    )
                    if last:
                        r0 = t * P
                        r1 = min((t + 1) * P, npc)
                        nc.sync.dma_start(
                            out=out_ext[r0:r1, :], in_=y[0 : r1 - r0, :]
                        )
                    else:
                        a_t = work.tile([P, dout], bf16, tag="a", name="a")
                        nc.scalar.activation(a_t[:], y[:], relu)
                        for kc in range(kcs_next):
                            wcols = min(P, dout - kc * P)
                            pt = psum.tile([wcols, P], bf16, tag="tr", name="tr")
                            nc.tensor.transpose(
                                pt[:], a_t[:, kc * P : kc * P + wcols], ident[:]
                            )
                            nc.scalar.copy(
                                aT_next[kc][:, t * P : (t + 1) * P], pt[:]
                            )
                        # fused next-layer local matmul + chunked AllGather
                        local_mm(li + 1, t, aT_next)
                        if t in fire_at:
                            fire_collective(li + 1, fire_at[t])
            if not last:
                aT = aT_next

    nc.compile()
    return nc


def _preprocess(x, edge_index, n_nodes, npc, npad, n_cores, dims):
    import ml_dtypes

    src = np.asarray(edge_index[0], dtype=np.int64)
    dst = np.asarray(edge_index[1], dtype=np.int64)
    deg = np.bincount(dst, minlength=n_nodes).astype(np.float32) + 1.0
    dinv = (1.0 / np.sqrt(deg)).astype(np.float32)

    ntiles = npad // P
    csplit = _chunk_splits(ntiles)
    cstart = [sum(csplit[:c]) for c in range(len(csplit))]
    crows = [
        min((cstart[c] + csplit[c]) * P, npc) - cstart[c] * P
        for c in range(len(csplit))
    ]
    nch = len(csplit)
    assert all(n_cores * r < 2**15 for r in crows), crows
    per_core = []
    for i in range(n_cores):
        lo = i * npc
        sel = (dst >= lo) & (dst < lo + npc)
        # self-loops are NOT materialized as gather tokens; the dinv^2 * h
        # self term is added in the epilogue from the local hs tile.
        s = src[sel]
        dr = dst[sel] - lo
        tl = dr // P
        d64 = (dr % P) // 64
        owner = s // npc
        rloc = s - owner * npc
        rtile = rloc // P
        ch = np.zeros(len(s), np.int64)
        for c in range(1, nch):
            ch[rtile >= cstart[c]] = c
        cs = np.array([cstart[c] * P for c in range(nch)])
        cr = np.array(crows)
        s = owner * cr[ch] + rloc - cs[ch]
        order = np.lexsort((s, d64, ch, tl))
        s, dr, ch, tl = s[order], dr[order], ch[order], tl[order]
        key = tl * nch + ch
        starts = np.searchsorted(key, np.arange(ntiles * nch), "left")
        ends = np.searchsorted(key, np.arange(ntiles * nch), "right")
        per_core.append((s, dr, starts, ends))

    seglen = np.zeros((ntiles, nch), np.int64)
    for s, dr, starts, ends in per_core:
        ln = (ends - starts).reshape(ntiles, nch)
        seglen = np.maximum(seglen, ln)
    seglen = ((seglen + P - 1) // P) * P

    # stream layout: per tile, one gather call per (chunk) segment
    layout = []
    pos = 0
    for g0 in range(0, ntiles, GROUP):
        tl_list = list(range(g0, min(g0 + GROUP, ntiles)))
        calls = []
        tiles = [[t, []] for t in tl_list]
        for h in range(nch):
            c0 = pos
            for k, t in enumerate(tl_list):
                L = int(seglen[t, h])
                if L:
                    tiles[k][1].append((h, pos // P, (pos + L) // P))
                pos += L
            if pos > c0:
                calls.append((h, c0, pos))
        layout.append({"calls": calls, "tiles": [(t, br) for t, br in tiles]})
    total_tok = pos

    in_maps = []
    drel_all = []
    for i in range(n_cores):
        s, dr, starts, ends = per_core[i]
        gidx = np.zeros(total_tok, np.int16)
        drel = np.full(total_tok, -1.0, np.float32)
        for grp in layout:
            for t, branges in grp["tiles"]:
                for h, j0, j1 in branges:
                    st, en = starts[t * nch + h], ends[t * nch + h]
                    n = en - st
                    p0 = j0 * P
                    gidx[p0 : p0 + n] = s[st:en].astype(np.int16)
                    drel[p0 : p0 + n] = (dr[st:en] - t * P).astype(np.float32)
        drel_all.append(drel)
        lo = i * npc
        x_loc = np.asarray(x[lo : lo + npc], dtype=np.float32)
        xT = np.zeros((x.shape[1], npad), dtype=ml_dtypes.bfloat16)
        xT[:, :npc] = x_loc.T.astype(ml_dtypes.bfloat16)
        dv = np.ones(npad, dtype=np.float32)
        dv[:npc] = dinv[lo : lo + npc]
        nbmax = 0
        in_maps.append(
            {
                "xT": xT,
                "ident": np.eye(P, dtype=ml_dtypes.bfloat16),
                "dinv": np.ascontiguousarray(dv.reshape(ntiles, P).T),
                "gidx": np.tile(
                    np.ascontiguousarray(gidx.reshape(total_tok // 16, 16).T),
                    (8, 1),
                ),
                "drel": np.ascontiguousarray(
                    drel.reshape(total_tok // P, P).T
                ).astype(ml_dtypes.bfloat16),
            }
        )

    # per-block needs-half flags, OR across cores (program is SPMD)
    nblk_tot = total_tok // P
    flags01 = np.zeros((nblk_tot, 2), bool)
    for drel in drel_all:
        blk = drel.reshape(nblk_tot, P)
        flags01[:, 0] |= ((blk >= 0) & (blk % P < 64)).any(axis=1)
        flags01[:, 1] |= ((blk >= 0) & (blk % P >= 64)).any(axis=1)
    flags01 = [tuple(bool(v) for v in row) for row in flags01]

    nbmax = max(
        (j1 - j0)
        for grp in layout
        for _, branges in grp["tiles"]
        for _, j0, j1 in branges
    )
    import ml_dtypes as mld

    for d in range(2):
        rio = np.broadcast_to(
            np.arange(d * 64, d * 64 + 64, dtype=np.float32), (P, nbmax, 64)
        ).astype(mld.bfloat16)
        for m in in_maps:
            m[f"riota{d}"] = rio
    return in_maps, layout, flags01, total_tok, dinv


def _pad_w(w, din_p, dout_p):
    out = np.zeros((din_p, dout_p), np.float32)
    out[: w.shape[0], : w.shape[1]] = w
    return out


def kernel(x, edge_index, W1, b1, W2, b2, W3, b3, W4, b4, **_unused):
    import ml_dtypes
    from concourse.bass_utils import run_bass_kernel_spmd

    x = np.asarray(x, dtype=np.float32)
    n_nodes = x.shape[0]
    npc = n_nodes // N_CORES
    ntiles = math.ceil(npc / P)
    npad = ntiles * P

    ws_raw = [np.asarray(w, np.float32) for w in (W1, W2, W3, W4)]
    bs_raw = [np.asarray(b, np.float32) for b in (b1, b2, b3, b4)]
    # pad every dim (except the first input / last output) to a multiple
    # of 128 so bf16 gather elem sizes stay multiples of 256B
    d_in = [ws_raw[0].shape[0]] + [
        max(P, math.ceil(w.shape[1] / P) * P) for w in ws_raw[:-1]
    ]
    d_last = max(P, math.ceil(ws_raw[-1].shape[1] / P) * P)
    d_out = d_in[1:] + [d_last]
    dims = list(zip(d_in, d_out))
    dout_raw = ws_raw[-1].shape[1]
    ws = [
        _pad_w(w, di, do).astype(ml_dtypes.bfloat16)
        for w, (di, do) in zip(ws_raw, dims)
    ]
    bs = [
        np.pad(b, (0, do - b.shape[0])).astype(np.float32)
        for b, (_, do) in zip(bs_raw, dims)
    ]

    in_maps, layout, flags01, total_tok, _ = _preprocess(
        x, edge_index, n_nodes, npc, npad, N_CORES, dims
    )
    key = (
        n_nodes,
        tuple(dims),
        total_tok,
        os.environ.get("GCN_SP", "1"),
        os.environ.get("GCN_QCYC", "1230"),
    )
    if key not in _CACHE:
        _CACHE[key] = _build_gcn(
            n_nodes, npc, npad, dims, layout, flags01, total_tok, N_CORES
        )
    nc = _CACHE[key]

    for m in in_maps:
        for i in range(4):
            m[f"w{i}"] = ws[i]
            m[f"b{i}"] = np.broadcast_to(bs[i], (P, bs[i].shape[0])).copy()

    if os.environ.get("GCN_SIM"):
        from concourse.bass_interp import MultiCoreSim

        sim = MultiCoreSim(nc, N_CORES)
        for i in range(N_CORES):
            for k, v in in_maps[i].items():
                sim.cores[i].tensor(k)[:] = v
        sim.simulate(check_with_hw=False)
        return np.concatenate(
            [sim.cores[i].mem_tensor("out") for i in range(N_CORES)], axis=0
        )[:, :dout_raw]

    trace = bool(os.environ.get("GCN_TRACE"))
    res = run_bass_kernel_spmd(
        nc, in_maps, core_ids=list(range(N_CORES)), trace=trace
    )
    global LAST_EXEC_NS
    LAST_EXEC_NS = res.exec_time_ns
    return np.concatenate(
        [res.results[i]["out"] for i in range(N_CORES)], axis=0
    )[:, :dout_raw]


# revision 29
# speedup vs baseline: 1.0893x; 1.0893x over previous
"""AnomalyDAE 4-layer GCN on 8 TRN2 NeuronCores.

Strategy (node sharding):
  - Nodes partitioned contiguously across 8 cores (6250/core).
  - Per layer: local matmul h = A @ W (PE, bf16 in / f32 acc), pre-scaled by
    dinv so symmetric normalization becomes out = dinv * segsum(hs[src]),
    hs = dinv * h.  AllGather the bf16 hs table -> [N, dout] per core.
  - Message passing: edges (incl. self-loops) sorted by (dst tile, table
    half, dst 64-half) on host.  Per 128-token block: dma_gather the src
    rows (bf16), build a 64-column one-hot Seg[tok, row64] =
    (dst_rel[tok] == iota) on DVE, and matmul Seg^T @ msg accumulates the
    tile's [64, dout] half-result in PSUM (rows 0:64 / 64:128 are two
    independent accumulation groups).  64-wide one-hot halves both DVE
    generation work and PE LDWEIGHTS columns vs. a 128-wide one.
  - Gather indices (int16, 16-partition wrap) and dst_rel are identical
    across layers: preloaded into SBUF once, no per-call index DMAs.
  - The next layer's local matmul is fused into the aggregation epilogue
    (relu -> PE-transpose -> matmul -> dinv scale -> hs dma).  AllGather
    half-A fires after tile 24's hs write (mid-layer), half-B after the
    last tile, so the collectives overlap the previous layer's tail and
    the gather queues never starve at layer boundaries.
  - int16 gather indices => the feature table is split in two halves (per
    core: rows [0,3200) -> table A, rest -> table B), each AllGathered
    separately.  The d=64 layer is zero-padded to 128 features (gather
    element size must be a multiple of 256 bytes).
"""

import math
import os

import numpy as np

N_CORES = 8
P = 128
GROUP = 1  # dst tiles per gather call pair

_CACHE = {}
LAST_EXEC_NS = None


def _build_gcn(n_nodes, npc, npad, dims, layout, flags01, total_tok, n_cores):
    import concourse.bacc as bacc
    import concourse.tile as tile
    from concourse import mybir
    from concourse.library_config import mlp
    from contextlib import ExitStack

    f32 = mybir.dt.float32
    bf16 = mybir.dt.bfloat16
    i16 = mybir.dt.int16
    ntiles = npad // P
    single_packet = os.environ.get("GCN_SP", "1") == "1"

    nc = bacc.Bacc(
        "TRN2", debug=False, num_devices=n_cores, num_swdge_queues=4
    )

    din0 = dims[0][0]
    kc0 = math.ceil(din0 / P)
    xT = nc.declare_dram_parameter("xT", [din0, npad], bf16, isOutput=False)
    Ws = [
        nc.declare_dram_parameter(f"w{i}", [din, dout], bf16, isOutput=False)
        for i, (din, dout) in enumerate(dims)
    ]
    Bs = [
        nc.declare_dram_parameter(f"b{i}", [P, dout], f32, isOutput=False)
        for i, (din, dout) in enumerate(dims)
    ]
    dinv_in = nc.declare_dram_parameter("dinv", [P, ntiles], f32, isOutput=False)
    ident_in = nc.declare_dram_parameter("ident", [P, P], bf16, isOutput=False)
    nbmax = max(
        (j1 - j0)
        for grp in layout
        for _, branges in grp["tiles"]
        for _, j0, j1 in branges
    )
    riota_in = [
        nc.declare_dram_parameter(f"riota{d}", [P, nbmax, 64], bf16, isOutput=False)
        for d in range(2)
    ]
    gi_in = nc.declare_dram_parameter(
        "gidx", [P, total_tok // 16], i16, isOutput=False
    )
    drel_in = nc.declare_dram_parameter(
        "drel", [P, total_tok // P], bf16, isOutput=False
    )
    dout_last = dims[-1][1]
    out_ext = nc.declare_dram_parameter("out", [npc, dout_last], f32, isOutput=True)

    split_t = (ntiles + 1) // 2
    rows_a = split_t * P            # per-core rows in half A (tile-aligned)
    rows_b = npc - rows_a           # per-core rows in half B
    hs_loc = [
        (
            nc.dram_tensor(f"hs_locA{i}", [rows_a, d], bf16),
            nc.dram_tensor(f"hs_locB{i}", [rows_b, d], bf16),
        )
        for i, (_, d) in enumerate(dims)
    ]
    hs_full = [
        (
            nc.dram_tensor(
                f"hs_fullA{i}", [n_cores * rows_a, d], bf16, addr_space="Shared"
            ),
            nc.dram_tensor(
                f"hs_fullB{i}", [n_cores * rows_b, d], bf16, addr_space="Shared"
            ),
        )
        for i, (_, d) in enumerate(dims)
    ]

    relu = mybir.ActivationFunctionType.Relu
    copyf = mybir.ActivationFunctionType.Copy
    mult = mybir.AluOpType.mult
    add = mybir.AluOpType.add
    iseq = mybir.AluOpType.is_equal

    n_layers = len(dims)

    with tile.TileContext(nc) as tc, ExitStack() as ctx:
        const = ctx.enter_context(tc.tile_pool(name="const", bufs=1))
        at_pool = ctx.enter_context(tc.tile_pool(name="acts", bufs=2))
        work = ctx.enter_context(tc.tile_pool(name="work", bufs=4))
        msgp = ctx.enter_context(tc.tile_pool(name="msg", bufs=6))
        segp = ctx.enter_context(tc.tile_pool(name="seg", bufs=10))
        psum = ctx.enter_context(tc.tile_pool(name="psum", bufs=2, space="PSUM"))
        psacc = ctx.enter_context(tc.tile_pool(name="psacc", bufs=4, space="PSUM"))

        nc.gpsimd.load_library(mlp)

        ident = const.tile([P, P], bf16)
        nc.sync.dma_start(out=ident[:], in_=ident_in[:, :])
        riota = []
        for d in range(2):
            rt = const.tile([P, nbmax, 64], bf16, tag=f"riota{d}")
            nc.sync.dma_start(out=rt[:], in_=riota_in[d][:, :, :])
            riota.append(rt)
        dinv_sb = const.tile([P, ntiles], f32)
        nc.sync.dma_start(out=dinv_sb[:], in_=dinv_in[:])
        drel_sb = const.tile([P, total_tok // P], bf16)
        nc.sync.dma_start(out=drel_sb[:], in_=drel_in[:, :])
        gidx_sb = const.tile([P, total_tok // 16], i16)
        nc.sync.dma_start(out=gidx_sb[:], in_=gi_in[:, :])

        w_sb, b_sb = [], []
        for i, (din, dout) in enumerate(dims):
            kcs = math.ceil(din / P)
            wi = []
            for kc in range(kcs):
                rows = min(P, din - kc * P)
                wt = const.tile([rows, dout], bf16, tag=f"w{i}_{kc}")
                nc.sync.dma_start(out=wt[:], in_=Ws[i][kc * P : kc * P + rows, :])
                wi.append(wt)
            w_sb.append(wi)
            bt = const.tile([P, dout], f32, tag=f"b{i}")
            nc.sync.dma_start(out=bt[:], in_=Bs[i][:, :])
            b_sb.append(bt)

        def local_mm(li, t, aT):
            """h = a @ W[li] for shard tile t, dinv-prescale, dma to hs_loc."""
            dout = dims[li][1]
            ps = psum.tile([P, dout], f32, tag="mm", name="mm")
            kcs = len(aT)
            for kc in range(kcs):
                nc.tensor.matmul(
                    ps[:],
                    aT[kc][:, t * P : (t + 1) * P],
                    w_sb[li][kc][:],
                    start=(kc == 0),
                    stop=(kc == kcs - 1),
                )
            hs_t = work.tile([P, dout], bf16, tag="hs", name="hs")
            nc.scalar.activation(
                hs_t[:], ps[:], copyf, bias=0.0, scale=dinv_sb[:, t : t + 1]
            )
            if t < split_t:
                r0 = t * P
                r1 = min((t + 1) * P, rows_a)
                dst = hs_loc[li][0]
            else:
                r0 = t * P - rows_a
                r1 = min((t + 1) * P - rows_a, rows_b)
                dst = hs_loc[li][1]
            nc.sync.dma_start(out=dst[r0:r1, :], in_=hs_t[0 : r1 - r0, :])

        def fire_collective(li, hh):
            nc.gpsimd.collective_compute(
                "AllGather",
                mybir.AluOpType.bypass,
                replica_groups=[list(range(n_cores))],
                ins=[hs_loc[li][hh][:, :]],
                outs=[hs_full[li][hh][:, :]],
            )

        # ---- layer-0 lead-in: load xT, local matmul per tile, AllGather ----
        aT = []
        for kc in range(kc0):
            rows = min(P, din0 - kc * P)
            t_ = at_pool.tile([rows, npad], bf16, tag=f"aT_{kc}", name=f"aT0_{kc}")
            nc.sync.dma_start(out=t_[:], in_=xT[kc * P : kc * P + rows, :])
            aT.append(t_)
        for t in range(ntiles):
            local_mm(0, t, aT)
            if t == split_t - 1:
                fire_collective(0, 0)
        fire_collective(0, 1)

        qn = 0
        for li, (din, dout) in enumerate(dims):
            last = li == n_layers - 1

            if not last:
                dnext = dims[li + 1][0]
                kcs_next = math.ceil(dnext / P)
                aT_next = []
                for kc in range(kcs_next):
                    rows = min(P, dnext - kc * P)
                    aT_next.append(
                        at_pool.tile(
                            [rows, npad],
                            bf16,
                            tag=f"aT_{kc}",
                            name=f"aT{li + 1}_{kc}",
                        )
                    )

            for grp in layout:
                msgs = {}
                for h, c0, c1 in grp["calls"]:
                    qn = (qn + 1) % 4
                    ntok = c1 - c0
                    msg = msgp.tile([P, ntok // P, dout], bf16, tag="msg", name="msg")
                    nc.gpsimd.dma_gather(
                        msg[:],
                        hs_full[li][h][:, :],
                        gidx_sb[:, c0 // 16 : c1 // 16],
                        ntok,
                        ntok,
                        dout,
                        single_packet=single_packet,
                        queue_num=qn,
                    )
                    msgs[h] = (msg, c0 // P)

                for t, branges in grp["tiles"]:
                    pacc = psacc.tile([P, dout], f32, tag="segacc", name="segacc")
                    # Per (tile, h) brange, half-0 blocks form a prefix and
                    # half-1 blocks a suffix (tokens sorted by dst 64-half);
                    # the straddle block appears in both.  Two independent
                    # PSUM accumulation groups: rows 0:64 (array cols 0:64)
                    # and 64:128 (cols 64:128).  MMs of the two groups are
                    # interleaved so each LDWEIGHTS targets the idle column
                    # strip and overlaps the other strip's MATMUL.
                    rngs = []  # (h, d64, b0, b1)
                    for h, j0, j1 in branges:
                        f0 = [j for j in range(j0, j1) if flags01[j][0]]
                        f1 = [j for j in range(j0, j1) if flags01[j][1]]
                        if f0:
                            rngs.append((h, 0, f0[0], f0[-1] + 1))
                        if f1:
                            rngs.append((h, 1, f1[0], f1[-1] + 1))
                    mmseq = [[], []]  # per d64: (seg_tile, blk_in_seg, h, j)
                    for h, d64, b0, b1 in rngs:
                        nblk = b1 - b0
                        sg = segp.tile([P, nblk, 64], bf16, tag="seg", name="seg")
                        nc.vector.tensor_tensor(
                            out=sg[:],
                            in0=drel_sb[:, b0:b1].to_broadcast([P, nblk, 64]),
                            in1=riota[d64][:, 0:nblk, :],
                            op=iseq,
                        )
                        for j in range(b0, b1):
                            mmseq[d64].append((sg, j - b0, h, j))
                    tot = [len(mmseq[0]), len(mmseq[1])]
                    nmm = [0, 0]
                    order = []
                    for k in range(max(tot)):
                        for d64 in range(2):
                            if k < tot[d64]:
                                order.append((d64, k))
                    for d64, k in order:
                        sg, bi, h, j = mmseq[d64][k]
                        msg, base = msgs[h]
                        nc.tensor.matmul(
                            pacc[d64 * 64 : d64 * 64 + 64, :],
                            sg[:, bi, :],
                            msg[:, j - base, :],
                            start=(nmm[d64] == 0),
                            stop=(nmm[d64] == tot[d64] - 1),
                            skip_group_check=True,
                        )
                        nmm[d64] += 1

                    # ---- epilogue straight from PSUM ----
                    y = work.tile([P, dout], f32, tag="y", name="y")
                    nc.vector.scalar_tensor_tensor(
                        y[:], pacc[:], dinv_sb[:, t : t + 1], b_sb[li][:], mult, add
                    )
                    if last:
                        r0 = t * P
                        r1 = min((t + 1) * P, npc)
                        nc.sync.dma_start(
                            out=out_ext[r0:r1, :], in_=y[0 : r1 - r0, :]
                        )
                    else:
                        a_t = work.tile([P, dout], bf16, tag="a", name="a")
                        nc.scalar.activation(a_t[:], y[:], relu)
                        for kc in range(kcs_next):
                            wcols = min(P, dout - kc * P)
                            pt = psum.tile([wcols, P], bf16, tag="tr", name="tr")
                            nc.tensor.transpose(
                                pt[:], a_t[:, kc * P : kc * P + wcols], ident[:]
                            )
                            nc.scalar.copy(
                                aT_next[kc][:, t * P : (t + 1) * P], pt[:]
                            )
                        # fused next-layer local matmul + chunked AllGather
                        local_mm(li + 1, t, aT_next)
                        if t == split_t - 1:
                            fire_collective(li + 1, 0)
                        elif t == ntiles - 1:
                            fire_collective(li + 1, 1)
            if not last:
                aT = aT_next

    nc.compile()
    return nc


def _preprocess(x, edge_index, n_nodes, npc, npad, n_cores, dims):
    import ml_dtypes

    src = np.asarray(edge_index[0], dtype=np.int64)
    dst = np.asarray(edge_index[1], dtype=np.int64)
    deg = np.bincount(dst, minlength=n_nodes).astype(np.float32) + 1.0
    dinv = (1.0 / np.sqrt(deg)).astype(np.float32)

    ntiles = npad // P
    split_t = (ntiles + 1) // 2
    rows_a = split_t * P
    rows_b = npc - rows_a
    per_core = []
    for i in range(n_cores):
        lo = i * npc
        sel = (dst >= lo) & (dst < lo + npc)
        s = np.concatenate([src[sel], np.arange(lo, lo + npc, dtype=np.int64)])
        dr = np.concatenate([dst[sel] - lo, np.arange(npc, dtype=np.int64)])
        tl = dr // P
        d64 = (dr % P) // 64
        owner = s // npc
        rloc = s - owner * npc
        hh = (rloc >= rows_a).astype(np.int64)
        s = np.where(hh == 0, owner * rows_a + rloc, owner * rows_b + rloc - rows_a)
        order = np.lexsort((s, d64, hh, tl))
        s, dr, hh, tl = s[order], dr[order], hh[order], tl[order]
        key = tl * 2 + hh
        starts = np.searchsorted(key, np.arange(ntiles * 2), "left")
        ends = np.searchsorted(key, np.arange(ntiles * 2), "right")
        per_core.append((s, dr, starts, ends))

    seglen = np.zeros((ntiles, 2), np.int64)
    for s, dr, starts, ends in per_core:
        ln = (ends - starts).reshape(ntiles, 2)
        seglen = np.maximum(seglen, ln)
    seglen = ((seglen + P - 1) // P) * P

    # stream layout: groups of GROUP tiles, within a group h0 segments then h1
    layout = []
    pos = 0
    for g0 in range(0, ntiles, GROUP):
        tl_list = list(range(g0, min(g0 + GROUP, ntiles)))
        calls = []
        tiles = [[t, []] for t in tl_list]
        for h in (0, 1):
            c0 = pos
            for k, t in enumerate(tl_list):
                L = int(seglen[t, h])
                if L:
                    tiles[k][1].append((h, pos // P, (pos + L) // P))
                pos += L
            if pos > c0:
                calls.append((h, c0, pos))
        layout.append({"calls": calls, "tiles": [(t, br) for t, br in tiles]})
    total_tok = pos

    in_maps = []
    drel_all = []
    for i in range(n_cores):
        s, dr, starts, ends = per_core[i]
        gidx = np.zeros(total_tok, np.int16)
        drel = np.full(total_tok, -1.0, np.float32)
        for grp in layout:
            for t, branges in grp["tiles"]:
                for h, j0, j1 in branges:
                    st, en = starts[t * 2 + h], ends[t * 2 + h]
                    n = en - st
                    p0 = j0 * P
                    gidx[p0 : p0 + n] = s[st:en].astype(np.int16)
                    drel[p0 : p0 + n] = (dr[st:en] - t * P).astype(np.float32)
        drel_all.append(drel)
        lo = i * npc
        x_loc = np.asarray(x[lo : lo + npc], dtype=np.float32)
        xT = np.zeros((x.shape[1], npad), dtype=ml_dtypes.bfloat16)
        xT[:, :npc] = x_loc.T.astype(ml_dtypes.bfloat16)
        dv = np.ones(npad, dtype=np.float32)
        dv[:npc] = dinv[lo : lo + npc]
        nbmax = 0
        in_maps.append(
            {
                "xT": xT,
                "ident": np.eye(P, dtype=ml_dtypes.bfloat16),
                "dinv": np.ascontiguousarray(dv.reshape(ntiles, P).T),
                "gidx": np.tile(
                    np.ascontiguousarray(gidx.reshape(total_tok // 16, 16).T),
                    (8, 1),
                ),
                "drel": np.ascontiguousarray(
                    drel.reshape(total_tok // P, P).T
                ).astype(ml_dtypes.bfloat16),
            }
        )

    # per-block needs-half flags, OR across cores (program is SPMD)
    nblk_tot = total_tok // P
    flags01 = np.zeros((nblk_tot, 2), bool)
    for drel in drel_all:
        blk = drel.reshape(nblk_tot, P)
        flags01[:, 0] |= ((blk >= 0) & (blk % P < 64)).any(axis=1)
        flags01[:, 1] |= ((blk >= 0) & (blk % P >= 64)).any(axis=1)
    flags01 = [tuple(bool(v) for v in row) for row in flags01]

    nbmax = max(
        (j1 - j0)
        for grp in layout
        for _, branges in grp["tiles"]
        for _, j0, j1 in branges
    )
    import ml_dtypes as mld

    for d in range(2):
        rio = np.broadcast_to(
            np.arange(d * 64, d * 64 + 64, dtype=np.float32), (P, nbmax, 64)
        ).astype(mld.bfloat16)
        for m in in_maps:
            m[f"riota{d}"] = rio
    return in_maps, layout, flags01, total_tok, dinv


def _pad_w(w, din_p, dout_p):
    out = np.zeros((din_p, dout_p), np.float32)
    out[: w.shape[0], : w.shape[1]] = w
    return out


def kernel(x, edge_index, W1, b1, W2, b2, W3, b3, W4, b4, **_unused):
    import ml_dtypes
    from concourse.bass_utils import run_bass_kernel_spmd

    x = np.asarray(x, dtype=np.float32)
    n_nodes = x.shape[0]
    npc = n_nodes // N_CORES
    ntiles = math.ceil(npc / P)
    npad = ntiles * P

    ws_raw = [np.asarray(w, np.float32) for w in (W1, W2, W3, W4)]
    bs_raw = [np.asarray(b, np.float32) for b in (b1, b2, b3, b4)]
    # pad every dim (except the first input / last output) to a multiple
    # of 128 so bf16 gather elem sizes stay multiples of 256B
    d_in = [ws_raw[0].shape[0]] + [
        max(P, math.ceil(w.shape[1] / P) * P) for w in ws_raw[:-1]
    ]
    d_last = max(P, math.ceil(ws_raw[-1].shape[1] / P) * P)
    d_out = d_in[1:] + [d_last]
    dims = list(zip(d_in, d_out))
    dout_raw = ws_raw[-1].shape[1]
    ws = [
        _pad_w(w, di, do).astype(ml_dtypes.bfloat16)
        for w, (di, do) in zip(ws_raw, dims)
    ]
    bs = [
        np.pad(b, (0, do - b.shape[0])).astype(np.float32)
        for b, (_, do) in zip(bs_raw, dims)
    ]

    in_maps, layout, flags01, total_tok, _ = _preprocess(
        x, edge_index, n_nodes, npc, npad, N_CORES, dims
    )
    key = (n_nodes, tuple(dims), total_tok, os.environ.get("GCN_SP", "1"))
    if key not in _CACHE:
        _CACHE[key] = _build_gcn(
            n_nodes, npc, npad, dims, layout, flags01, total_tok, N_CORES
        )
    nc = _CACHE[key]

    for m in in_maps:
        for i in range(4):
            m[f"w{i}"] = ws[i]
            m[f"b{i}"] = np.broadcast_to(bs[i], (P, bs[i].shape[0])).copy()

    if os.environ.get("GCN_SIM"):
        from concourse.bass_interp import MultiCoreSim

        sim = MultiCoreSim(nc, N_CORES)
        for i in range(N_CORES):
            for k, v in in_maps[i].items():
                sim.cores[i].tensor(k)[:] = v
        sim.simulate(check_with_hw=False)
        return np.concatenate(
            [sim.cores[i].mem_tensor("out") for i in range(N_CORES)], axis=0
        )[:, :dout_raw]

    trace = bool(os.environ.get("GCN_TRACE"))
    res = run_bass_kernel_spmd(
        nc, in_maps, core_ids=list(range(N_CORES)), trace=trace
    )
    global LAST_EXEC_NS
    LAST_EXEC_NS = res.exec_time_ns
    return np.concatenate(
        [res.results[i]["out"] for i in range(N_CORES)], axis=0
    )[:, :dout_raw]
